# revision 1
# baseline (speedup 1.0000x reference)
"""Trainium2 Bass kernel for nn_MAMLAwareGANLoss.

Reference computation (B=1024, Z=256, H=W=128, N=H*W=16384):
    fake   = tanh(noise @ Wg)                      # [B, N]
    d_fake = fake @ Wd                             # [B, 1]
    g_loss = mean(softplus(-d_fake))               # (+ 0.0 * sum(d_real) == 0)
    solvability_loss = mean(per-sample flood-fill penalty of (fake == 1.0) walls)
    cur    = mean(fake == 1.0)
    difficulty_loss  = (cur - current_difficulty)^2
    loss   = g_loss + w_s * solvability_loss + w_d * difficulty_loss

Key structural facts used here:
  * real_mazes enters only through `0.0 * sum(d_real)` == exactly 0.0 -> never loaded.
  * "walls" are cells where float32 tanh(x) rounds to exactly 1.0, which requires
    x >= ~9.01.  We prove on the host (Cauchy-Schwarz over the actual inputs:
    max_b ||noise_b|| * max_n ||Wg[:, n]||) that no |x| can exceed the threshold,
    hence wall count == 0 exactly => solvability_loss == 0.0 and cur == 0.0.
    If the bound ever fails we fall back to an exact host recomputation.
  * Therefore the device only computes d_fake = (tanh(noise @ Wg)) @ Wd.

Device sharding (8 cores): shard the N (=H*W) dimension, 2048 columns/core.
Each core computes, for all 1024 samples, the partial dot product
    dpart[b] = sum_{n in shard} tanh((noise @ Wg)[b, n]) * Wd[n]
The host sums the 8 partials, applies softplus and the scalar tail.

Per-core device program (layout: n on PSUM partitions, b on free axis):
    x[n, b]  = sum_z Wg[z, n] * noiseT[z, b]    (PE, fp32r, K=z on partitions)
    t[n, b]  = tanh(x[n, b])                     (ACT, PSUM->SBUF)
    dpart[b] = sum_n Wd[n] * t[n, b]             (PE, stationary = Wd column)
This keeps the weighted reduction on the tensor engine (contraction over
partitions), which is far faster than DVE fp32 reductions.
"""

import numpy as np
import ml_dtypes

B, Z, H, W = 1024, 256, 128, 128
N = H * W               # 16384
NCORES = 8
NSH = N // NCORES       # 2048 columns of Wg per core
P = 128
NT = NSH // P           # 16 n-tiles per core
NB = B                  # 1024 samples (free axis)
CHUNKS = 2              # Wg DMA chunks per core
CW = NSH // CHUNKS      # 1024 columns per chunk
TILES_PER_CHUNK = NT // CHUNKS

# float32 tanh(x) rounds to exactly 1.0 only for x >= ~9.01; stay well below.
WALL_SAFE_BOUND = 8.5

_PROG = None  # cached compiled Bass program


def _build_program():
    import concourse.bass as bass
    import concourse.tile as tile
    from concourse import bacc, mybir

    f32 = mybir.dt.float32
    f32r = mybir.dt.float32r
    Tanh = mybir.ActivationFunctionType.Tanh

    nc = bacc.Bacc(
        "TRN2", target_bir_lowering=False, debug=False, num_devices=NCORES
    )
    # Inputs are host-relaid so every DMA source is one contiguous block:
    #   noise_t : [2(z), 128, 1024]       = noise.T z-planes (4KB lines)
    #   wg_shard: [CHUNKS, 2(z), 128, CW] = Wg shard tiled
    bf16 = mybir.dt.bfloat16
    noise_d = nc.declare_dram_parameter(
        "noise_t", [P, 2, NB], bf16, isOutput=False
    )
    wg_d = nc.declare_dram_parameter(
        "wg_shard", [CHUNKS, P, 2, CW], bf16, isOutput=False
    )
    wd_d = nc.declare_dram_parameter("wd_shard", [P, NT], bf16, isOutput=False)
    # Four partial rows (one per PE column-group used by the reduction);
    # the host sums them.
    out_d = nc.declare_dram_parameter("dpart", [4, NB], f32, isOutput=True)

    with tile.TileContext(nc) as tc:
        with (
            tc.tile_pool(name="const", bufs=1) as cpool,
            tc.tile_pool(name="wg", bufs=1) as wgpool,
            tc.tile_pool(name="t", bufs=3) as tpool,
            tc.tile_pool(name="ps", bufs=3, space="PSUM") as pspool,
            tc.tile_pool(name="dps", bufs=1, space="PSUM") as dpool,
        ):
            # Reduction accumulators (one PSUM bank each): the reduce matmul
            # for n-tile i runs in PE column-group i%4 (tile_position=
            # (0, 32*(i%4))), accumulating into partition 32*(i%4).  Four
            # reduce matmuls in distinct column groups execute concurrently.
            dd0 = dpool.tile([P, 512], f32, tag="dd0")
            dd1 = dpool.tile([P, 512], f32, tag="dd1")
            dd = [dd0, dd1]

            # --- PE warm-up: HAM-unthrottle the tensor engine during the DMA
            # wait using matmuls on a memset tile (no DMA dependency).
            # Plain fp32 matmuls run at quarter rate, so a handful keeps the
            # PE busy for the ~3.4us HAM window.  Output goes to dd, which
            # the real reduction later clears with start=True.
            warm_sb = cpool.tile([P, 512], f32, tag="warm")
            nc.gpsimd.memset(warm_sb[:], 0.0)
            # Preload the tanh activation table (~2.7us) during the DMA wait
            # instead of right before the first real tanh.
            warm_act = cpool.tile([P, 16], f32, tag="warm_act")
            nc.scalar.activation(warm_act[:], warm_sb[:, 0:16], Tanh)
            for _ in range(8):
                nc.tensor.matmul(
                    dd0[0:1, 0:256],
                    warm_sb[:, 0:1],
                    warm_sb[:, 0:256],
                    start=True,
                    stop=True,
                    skip_group_check=True,
                )

            # DMA issue spread across three engine queues; the blocks that
            # gate the first matmuls are placed at the head of each queue.
            wg_chunks = [
                wgpool.tile([P, 2, CW], bf16, name=f"wg{ci}", tag=f"wg{ci}")
                for ci in range(CHUNKS)
            ]
            noise_sb = cpool.tile([P, 2, NB], bf16, tag="noise")
            wd_sb = cpool.tile([P, NT], bf16, tag="wd")

            # Queue plan: scalar gets noise z0 (gates MM #1) + wd; sync gets
            # wg z0 chunks + noise z1; gpsimd gets wg z1 chunks.
            nc.gpsimd.dma_start(out=noise_sb[:, 0:1, :], in_=noise_d[:, 0:1, :])
            nc.scalar.dma_start(out=noise_sb[:, 1:2, :], in_=noise_d[:, 1:2, :])
            nc.sync.dma_start(out=wg_chunks[0][:], in_=wg_d[0])
            nc.scalar.dma_start(out=wd_sb[:], in_=wd_d[:])
            nc.gpsimd.dma_start(out=wg_chunks[1][:], in_=wg_d[1])

            def emit_reduce_quad(group, t_tiles):
                # 4 n-tiles' reductions in 4 distinct PE column groups; they
                # issue back-to-back and run concurrently on the array.
                for h in range(2):
                    for j in range(4):
                        i = group * 4 + j
                        nc.tensor.matmul(
                            dd[h][32 * j : 32 * j + 1, :],
                            wd_sb[:, i : i + 1],
                            t_tiles[j][:, h * 512 : (h + 1) * 512],
                            start=(group == 0),
                            stop=(group == 3),
                            tile_position=(0, 32 * j),
                            skip_group_check=True,
                        )

            # Reduce quads lag the main matmuls by one group of 4 n-tiles so
            # the PE never stalls waiting for the group's last tanh.
            group_ts = []
            for i in range(NT):
                ci, sub = divmod(i, TILES_PER_CHUNK)
                wt = wg_chunks[ci]
                ps = pspool.tile([P, NB], f32)
                t = tpool.tile([P, NB], bf16, name=f"t{i}", tag="t", bufs=8)
                if i < 2:
                    # Fine-grained ordering for the first tiles: tanh starts
                    # after two small matmuls instead of four big ones,
                    # pulling the whole ACT-paced stream earlier.
                    nq = 4 if i == 0 else 2
                    qw = NB // nq
                    for q in range(nq):
                        for z in range(2):
                            nc.tensor.matmul(
                                ps[:, q * qw : (q + 1) * qw],
                                wt[:, z : z + 1, sub * P : (sub + 1) * P],
                                noise_sb[:, z : z + 1, q * qw : (q + 1) * qw],
                                start=(z == 0),
                                stop=(z == 1),
                            )
                        nc.scalar.activation(
                            t[:, q * qw : (q + 1) * qw],
                            ps[:, q * qw : (q + 1) * qw],
                            Tanh,
                        )
                else:
                    for z in range(2):
                        wg_blk = wt[:, z : z + 1, sub * P : (sub + 1) * P]
                        for h in range(2):
                            nc.tensor.matmul(
                                ps[:, h * 512 : (h + 1) * 512],
                                wg_blk,
                                noise_sb[:, z : z + 1, h * 512 : (h + 1) * 512],
                                start=(z == 0),
                                stop=(z == 1),
                            )
                    nc.scalar.activation(t[:], ps[:], Tanh)
                group_ts.append(t)
                if i % 4 == 3 and i >= 7:
                    g = i // 4 - 1
                    emit_reduce_quad(g, group_ts[g * 4 : g * 4 + 4])
            emit_reduce_quad(3, group_ts[12:16])

            out_sb = cpool.tile([97, NB], f32, tag="out")
            nc.scalar.copy(out_sb[:, 0:512], dd0[0:97, :])
            nc.vector.tensor_copy(out_sb[:, 512:1024], dd1[0:97, :])
            nc.sync.dma_start(out=out_d[:], in_=out_sb[0:97:32, :])

    nc.compile()
    return nc


def _get_program():
    global _PROG
    if _PROG is None:
        _PROG = _build_program()
    return _PROG


def _make_in_maps(noise, Wg, Wd):
    # noise.T as [128, 2(z), 1024] with z interleaved inside each partition
    # (4KB contiguous lines on both DMA sides)
    noise_t = np.ascontiguousarray(
        noise.T.astype(ml_dtypes.bfloat16).reshape(2, P, NB).transpose(1, 0, 2)
    )
    in_maps = []
    for c in range(NCORES):
        wg_c = Wg[:, c * NSH : (c + 1) * NSH]  # [Z, NSH]
        # -> contiguous [chunk, z, 128, CW] blocks
        wg_t = np.ascontiguousarray(
            wg_c.astype(ml_dtypes.bfloat16)
            .reshape(2, P, CHUNKS, CW)
            .transpose(2, 1, 0, 3)
        )
        seg = Wd[c * NSH : (c + 1) * NSH, 0]
        wd_c = np.ascontiguousarray(seg.reshape(NT, P).T).astype(ml_dtypes.bfloat16)
        in_maps.append({"noise_t": noise_t, "wg_shard": wg_t, "wd_shard": wd_c})
    return in_maps


def run_device(noise, Wg, Wd, trace=False):
    """Run the SPMD kernel on 8 cores; return (d_fake[B] float64, results)."""
    from concourse.bass_utils import run_bass_kernel_spmd

    nc = _get_program()
    in_maps = _make_in_maps(noise, Wg, Wd)
    res = run_bass_kernel_spmd(nc, in_maps, list(range(NCORES)), trace=trace)
    d_fake = np.zeros(NB, np.float64)
    for r in res.results:
        d_fake += np.asarray(r["dpart"], np.float64).reshape(4, NB).sum(axis=0)
    return d_fake, res


def _dilate(v):
    out = v.copy()
    out[:-1, :] |= v[1:, :]
    out[1:, :] |= v[:-1, :]
    out[:, :-1] |= v[:, 1:]
    out[:, 1:] |= v[:, :-1]
    return out


def _host_exact_maze_terms(noise, Wg):
    """Fallback (practically unreachable): exact wall/flood-fill computation."""
    solv = 0.0
    wall_total = 0
    for b0 in range(0, B, 64):
        x = noise[b0 : b0 + 64].astype(np.float32) @ Wg.astype(np.float32)
        fake = np.tanh(x).astype(np.float32)
        for j in range(fake.shape[0]):
            maze = fake[j].reshape(H, W)
            wall = maze == np.float32(1.0)
            nwall = int(wall.sum())
            wall_total += nwall
            pen = 0.0
            if float(wall.mean()) > 0.5:
                pen += 1.0
            if nwall >= 3:
                open_ = ~wall
                visited = np.zeros((H, W), bool)
                visited[1, 1] = True
                while True:
                    nv = visited | (_dilate(visited) & open_)
                    if not (nv & ~visited).any():
                        break
                    visited = nv
                wf = wall.astype(np.float32)
                wa = np.zeros((H, W), np.float32)
                wa[:-1, :] += wf[1:, :]
                wa[1:, :] += wf[:-1, :]
                wa[:, :-1] += wf[:, 1:]
                wa[:, 1:] += wf[:, :-1]
                pen += 0.1 * float((visited & (wa >= 3.0)).sum())
            solv += pen
    solv /= B
    cur = wall_total / float(B * H * W)
    return solv, cur


def kernel(**inputs) -> np.ndarray:
    noise = np.asarray(inputs["noise"], np.float32)
    Wg = np.asarray(inputs["Wg"], np.float32)
    Wd = np.asarray(inputs["Wd"], np.float32)
    p = float(np.asarray(inputs["maml_performance"]).reshape(-1)[0])
    cd = float(np.asarray(inputs["current_difficulty"]).reshape(-1)[0])

    d_fake, _ = run_device(noise, Wg, Wd)

    # g_loss = mean(softplus(-d_fake));  0.0 * sum(d_real) == 0 exactly.
    g_loss = float(np.mean(np.logaddexp(0.0, -d_fake)))

    # Wall existence bound: |x[b,n]| <= max_b||noise_b|| * max_n||Wg[:,n]||.
    rn = float(np.sqrt((noise.astype(np.float64) ** 2).sum(axis=1)).max())
    cn = float(np.sqrt((Wg.astype(np.float64) ** 2).sum(axis=0)).max())
    if rn * cn * 1.0001 < WALL_SAFE_BOUND:
        solv, cur = 0.0, 0.0
    else:  # pragma: no cover - requires |pre-tanh| ~ 28 sigma
        solv, cur = _host_exact_maze_terms(noise, Wg)

    w_s = 0.8 if p < 0.4 else (0.4 if p > 0.6 else 0.6)
    w_d = 0.05 if p < 0.4 else (0.2 if p > 0.6 else 0.1)
    difficulty = (cur - cd) ** 2
    loss = g_loss + w_s * solv + w_d * difficulty
    return np.array(loss, dtype=np.float32)

